# revision 1
# baseline (speedup 1.0000x reference)
"""Trainium2 Bass kernel for scaled dot-product attention.

Problem: B=16, L=S=2048, DK=DV=128, fp32.
reference: scores = (Q @ K^T) * 1/sqrt(DK); attn = softmax(scores, -1);
           out = attn @ V; returns (out, attn).

Sharding: data-parallel over batch, 2 batches per core on 8 NeuronCores.

Per-batch plan (all fp32):
  - Load Q, K natural [128, nT, 128]; PE-transpose each 128x128 tile to build
    QT, KT [128(d), L] in SBUF (contraction dim on partitions).
  - Phase A (per 128-row l_tile): scores[l,s] via matmul(lhsT=QT tile,
    rhs=KT) -> PSUM; ACT exp(scale*x) with fused accum_out row-sums;
    DVE reciprocal; DVE tensor_scalar normalize (2x mode); contiguous DMA
    store of attn rows.
  - Phase B (per 512-col l_chunk): scoresT[s,l] via matmul(lhsT=KT tile,
    rhs=QT chunk); ACT exp; PV matmul with V stationary accumulating
    outT[v, l] in PSUM over all 16 s-tiles; PE-transpose outT back to
    natural layout and normalize with phase-A's 1/rowsum.
"""

import math

import numpy as np

import concourse.bass as bass
import concourse.tile as tile
from concourse import bacc, mybir
from concourse.bass import ds, ts
from concourse.masks import make_identity

FP32 = mybir.dt.float32
P = 128
B_FULL = 16
N_CORES = 8
B_PER_CORE = B_FULL // N_CORES
L_FULL = 2048
S_FULL = 2048
DK = 128
DV = 128


def build_attention_body(tc, q, k, v, attn, out):
    nc = tc.nc
    n_b, Lb, _ = q.shape
    Sb = k.shape[1]
    scale = 1.0 / math.sqrt(DK)

    n_lt = Lb // P          # 128-row l tiles
    n_st = Sb // P          # 128-row s tiles
    n_lc = Lb // 512        # 512-col l chunks (phase B)
    a_half = min(1024, Sb)  # phase A PSUM tile width
    n_ah = Sb // a_half
    n_sp = (n_st + 1) // 2  # phase B processes s tiles in pairs

    with (
        tc.tile_pool(name="consts", bufs=1) as consts,
        tc.tile_pool(name="p_nat", bufs=2) as p_nat,
        tc.tile_pool(name="p_qkT", bufs=2) as p_qkT,
        tc.tile_pool(name="p_expA", bufs=3) as p_expA,
        tc.tile_pool(name="p_attn", bufs=3) as p_attn,
        tc.tile_pool(name="p_expB", bufs=3) as p_expB,
        tc.tile_pool(name="p_small", bufs=2) as p_small,
        tc.tile_pool(name="p_stat", bufs=24) as p_stat,
        tc.tile_pool(name="ps_big", bufs=3, space="PSUM") as ps_big,
        tc.tile_pool(name="ps_small", bufs=2, space="PSUM") as ps_small,
    ):
        identity = consts.tile([P, P], FP32)
        make_identity(nc, identity[:])

        for b in range(n_b):
            # ---- load + build transposed QT/KT, natural V ----
            q_nat = p_nat.tile([P, n_lt, DK], FP32, tag="qnat")
            nc.sync.dma_start(q_nat[:], q[b].rearrange("(t p) d -> p t d", p=P))
            k_nat = p_nat.tile([P, n_st, DK], FP32, tag="knat")
            nc.sync.dma_start(k_nat[:], k[b].rearrange("(t p) d -> p t d", p=P))
            v_sb = p_nat.tile([P, n_st, DV], FP32, tag="vsb")
            nc.sync.dma_start(v_sb[:], v[b].rearrange("(t p) d -> p t d", p=P))

            qT = p_qkT.tile([P, Lb], FP32, tag="qT")
            for t in range(n_lt):
                tr = ps_small.tile([P, 512], FP32, tag="pss")
                nc.tensor.transpose(tr[:, 0:P], q_nat[:, t, :], identity[:])
                nc.vector.tensor_copy(qT[:, ts(t, P)], tr[:, 0:P])
            kT = p_qkT.tile([P, Sb], FP32, tag="kT")
            for t in range(n_st):
                tr = ps_small.tile([P, 512], FP32, tag="pss")
                nc.tensor.transpose(tr[:, 0:P], k_nat[:, t, :], identity[:])
                nc.vector.tensor_copy(kT[:, ts(t, P)], tr[:, 0:P])

            inv_tiles = {}
            for c in range(n_lc):
                # ---- phase A: natural-layout scores/softmax for l tiles ----
                for t4 in range(4):
                    t = 4 * c + t4
                    expA = p_expA.tile([P, Sb], FP32, tag="expA")
                    sum_parts = []
                    for h in range(n_ah):
                        psA = ps_big.tile([P, a_half], FP32, tag="big")
                        for j in range(a_half // 512):
                            nc.tensor.matmul(
                                psA[:, ds(512 * j, 512)],
                                qT[:, ts(t, P)],
                                kT[:, ds(a_half * h + 512 * j, 512)],
                                start=True,
                                stop=True,
                            )
                        sum_h = p_stat.tile([P, 1], FP32, tag="sumh")
                        nc.scalar.activation(
                            expA[:, ds(a_half * h, a_half)],
                            psA[:],
                            mybir.ActivationFunctionType.Exp,
                            scale=scale,
                            accum_out=sum_h[:],
                        )
                        sum_parts.append(sum_h)
                    inv_t = p_stat.tile([P, 1], FP32, tag="inv")
                    if n_ah == 1:
                        nc.vector.reciprocal(inv_t[:], sum_parts[0][:])
                    else:
                        sum_t = p_stat.tile([P, 1], FP32, tag="sumt")
                        nc.vector.tensor_add(sum_t[:], sum_parts[0][:], sum_parts[1][:])
                        nc.vector.reciprocal(inv_t[:], sum_t[:])
                    inv_tiles[t] = inv_t
                    attn_sb = p_attn.tile([P, Sb], FP32, tag="attn")
                    nc.vector.tensor_scalar_mul(attn_sb[:], expA[:], inv_t[:])
                    nc.sync.dma_start(attn[b][ds(P * t, P), :], attn_sb[:])

                # ---- phase B: transposed scores + PV matmul for this chunk ----
                outT = ps_small.tile([P, 512], FP32, tag="pss")
                for sp in range(n_sp):
                    psT = ps_big.tile([P, 1024], FP32, tag="big")
                    expB = p_expB.tile([P, 1024], FP32, tag="expB")
                    for i in range(2):
                        s = 2 * sp + i
                        nc.tensor.matmul(
                            psT[:, ds(512 * i, 512)],
                            kT[:, ts(s, P)],
                            qT[:, ds(512 * c, 512)],
                            start=True,
                            stop=True,
                        )
                    nc.scalar.activation(
                        expB[:],
                        psT[:],
                        mybir.ActivationFunctionType.Exp,
                        scale=scale,
                    )
                    for i in range(2):
                        s = 2 * sp + i
                        nc.tensor.matmul(
                            outT[:],
                            v_sb[:, s, :],
                            expB[:, ds(512 * i, 512)],
                            start=(s == 0),
                            stop=(s == n_st - 1),
                        )
                outT_sb = p_small.tile([P, 512], FP32, tag="outTsb")
                nc.vector.tensor_copy(outT_sb[:], outT[:])
                out_c = p_small.tile([P, 4, DV], FP32, tag="outc")
                for t4 in range(4):
                    tr = ps_small.tile([P, 512], FP32, tag="pss")
                    nc.tensor.transpose(tr[:, 0:P], outT_sb[:, ts(t4, P)], identity[:])
                    nc.vector.tensor_scalar_mul(
                        out_c[:, t4, :], tr[:, 0:P], inv_tiles[4 * c + t4][:]
                    )
                nc.sync.dma_start(
                    out[b].rearrange("(t p) d -> p t d", p=P)[:, ds(4 * c, 4), :],
                    out_c[:],
                )


def build_module(n_b=B_PER_CORE, Lb=L_FULL, Sb=S_FULL, enable_asserts=False):
    nc = bacc.Bacc(
        "TRN2",
        target_bir_lowering=False,
        debug=False,
        enable_asserts=enable_asserts,
        num_devices=N_CORES,
    )
    q = nc.dram_tensor("q", (n_b, Lb, DK), FP32, kind="ExternalInput").ap()
    k = nc.dram_tensor("k", (n_b, Sb, DK), FP32, kind="ExternalInput").ap()
    v = nc.dram_tensor("v", (n_b, Sb, DV), FP32, kind="ExternalInput").ap()
    attn = nc.dram_tensor("attn", (n_b, Lb, Sb), FP32, kind="ExternalOutput").ap()
    out = nc.dram_tensor("out", (n_b, Lb, DV), FP32, kind="ExternalOutput").ap()
    with tile.TileContext(nc) as tc:
        build_attention_body(tc, q, k, v, attn, out)
    nc.compile()
    return nc


_CACHED_NC = None


def _get_module():
    global _CACHED_NC
    if _CACHED_NC is None:
        _CACHED_NC = build_module()
    return _CACHED_NC


def kernel(query, key, value):
    from concourse.bass_utils import run_bass_kernel_spmd

    query = np.ascontiguousarray(np.asarray(query, dtype=np.float32))
    key = np.ascontiguousarray(np.asarray(key, dtype=np.float32))
    value = np.ascontiguousarray(np.asarray(value, dtype=np.float32))

    nc = _get_module()
    in_maps = []
    for core in range(N_CORES):
        sl = slice(core * B_PER_CORE, (core + 1) * B_PER_CORE)
        in_maps.append({"q": query[sl], "k": key[sl], "v": value[sl]})
    res = run_bass_kernel_spmd(nc, in_maps, core_ids=list(range(N_CORES)))
    out = np.concatenate([r["out"] for r in res.results], axis=0)
    attn = np.concatenate([r["attn"] for r in res.results], axis=0)
    return out, attn


# revision 3
# speedup vs baseline: 1.9638x; 1.9638x over previous
"""Trainium2 Bass kernel for scaled dot-product attention.

Problem: B=16, L=S=2048, DK=DV=128, fp32.
reference: scores = (Q @ K^T) * 1/sqrt(DK); attn = softmax(scores, -1);
           out = attn @ V; returns (out, attn).

Sharding: data-parallel over batch, 2 batches per core on 8 NeuronCores.

Per-batch plan:
  - Load Q, K natural [128, nT, 128]; round to fp32r; PE-transpose each
    128x128 tile to build QT, KT [128(d), L] in SBUF (contraction dim on
    partitions).
  - Phase A (per 128-row l_tile): scores[l,s] via matmul(lhsT=QT tile,
    rhs=KT) -> PSUM; ACT exp(scale*x) with fused accum_out row-sums;
    DVE reciprocal; DVE tensor_scalar normalize (2x mode); contiguous DMA
    store of attn rows.
  - Phase B (per 512-col l_chunk): scoresT[s,l] via matmul(lhsT=KT tile,
    rhs=QT chunk); ACT exp; PV matmul with V stationary accumulating
    outT[v, l] in PSUM over all 16 s-tiles; PE-transpose outT back to
    natural layout and normalize with phase-A's 1/rowsum.

MM_DTYPE fp32r: matmuls run at 1 cycle/row (vs 4 for fp32) with tf32-like
input rounding (~1.6e-4 matmul rel err).
"""

import math

import numpy as np

import concourse.bass as bass
import concourse.tile as tile
from concourse import bacc, mybir
from concourse.bass import ds, ts
from concourse.masks import make_identity

FP32 = mybir.dt.float32
FP32R = mybir.dt.float32r
P = 128
B_FULL = 16
N_CORES = 8
B_PER_CORE = B_FULL // N_CORES
L_FULL = 2048
S_FULL = 2048
DK = 128
DV = 128

MM_DTYPE = FP32R  # FP32 for exact (4x slower matmuls), FP32R for fast


def build_attention_body(tc, q, k, v, attn, out, mm_dtype=MM_DTYPE):
    nc = tc.nc
    n_b, Lb, _ = q.shape
    Sb = k.shape[1]
    scale = 1.0 / math.sqrt(DK)

    n_lt = Lb // P          # 128-row l tiles
    n_st = Sb // P          # 128-row s tiles
    n_lc = Lb // 512        # 512-col l chunks (phase B)
    a_half = min(1024, Sb)  # phase A PSUM tile width
    n_ah = Sb // a_half
    n_sp = (n_st + 1) // 2  # phase B processes s tiles in pairs

    rounded = mm_dtype != FP32

    with (
        tc.tile_pool(name="consts", bufs=1) as consts,
        tc.tile_pool(name="p_nat", bufs=2) as p_nat,
        tc.tile_pool(name="p_qkT", bufs=2) as p_qkT,
        tc.tile_pool(name="p_expA", bufs=3) as p_expA,
        tc.tile_pool(name="p_attn", bufs=3) as p_attn,
        tc.tile_pool(name="p_expB", bufs=3) as p_expB,
        tc.tile_pool(name="p_small", bufs=2) as p_small,
        tc.tile_pool(name="p_stat", bufs=24) as p_stat,
        tc.tile_pool(name="ps_big", bufs=3, space="PSUM") as ps_big,
        tc.tile_pool(name="ps_small", bufs=2, space="PSUM") as ps_small,
    ):
        identity32 = consts.tile([P, P], FP32, tag="id32")
        make_identity(nc, identity32[:])
        if mm_dtype != FP32:
            identity = consts.tile([P, P], mm_dtype, tag="idr")
            nc.vector.tensor_copy(identity[:], identity32[:])
        else:
            identity = identity32

        for b in range(n_b):
            # ---- load; round to mm dtype; build transposed QT/KT; natural V ----
            q_nat = p_nat.tile([P, n_lt, DK], FP32, tag="qnat")
            nc.sync.dma_start(q_nat[:], q[b].rearrange("(t p) d -> p t d", p=P))
            k_nat = p_nat.tile([P, n_st, DK], FP32, tag="knat")
            nc.sync.dma_start(k_nat[:], k[b].rearrange("(t p) d -> p t d", p=P))
            v_sb = p_nat.tile([P, n_st, DV], FP32, tag="vsb")
            nc.sync.dma_start(v_sb[:], v[b].rearrange("(t p) d -> p t d", p=P))

            if rounded:
                q_r = p_nat.tile([P, n_lt, DK], mm_dtype, tag="qr")
                nc.vector.tensor_copy(q_r[:], q_nat[:])
                k_r = p_nat.tile([P, n_st, DK], mm_dtype, tag="kr")
                nc.vector.tensor_copy(k_r[:], k_nat[:])
                v_r = p_nat.tile([P, n_st, DV], mm_dtype, tag="vr")
                nc.vector.tensor_copy(v_r[:], v_sb[:])
            else:
                q_r, k_r, v_r = q_nat, k_nat, v_sb

            qT = p_qkT.tile([P, Lb], mm_dtype, tag="qT")
            for t in range(n_lt):
                tr = ps_small.tile([P, 512], mm_dtype, tag="pss")
                nc.tensor.transpose(tr[:, 0:P], q_r[:, t, :], identity[:])
                nc.vector.tensor_copy(qT[:, ts(t, P)], tr[:, 0:P])
            kT = p_qkT.tile([P, Sb], mm_dtype, tag="kT")
            for t in range(n_st):
                tr = ps_small.tile([P, 512], mm_dtype, tag="pss")
                nc.tensor.transpose(tr[:, 0:P], k_r[:, t, :], identity[:])
                nc.vector.tensor_copy(kT[:, ts(t, P)], tr[:, 0:P])

            inv_tiles = {}
            for c in range(n_lc):
                # ---- phase A: natural-layout scores/softmax for l tiles ----
                for t4 in range(4):
                    t = 4 * c + t4
                    expA = p_expA.tile([P, Sb], FP32, tag="expA")
                    sum_parts = []
                    for h in range(n_ah):
                        psA = ps_big.tile([P, a_half], FP32, tag="big")
                        for j in range(a_half // 512):
                            nc.tensor.matmul(
                                psA[:, ds(512 * j, 512)],
                                qT[:, ts(t, P)],
                                kT[:, ds(a_half * h + 512 * j, 512)],
                                start=True,
                                stop=True,
                            )
                        sum_h = p_stat.tile([P, 1], FP32, tag="sumh")
                        nc.scalar.activation(
                            expA[:, ds(a_half * h, a_half)],
                            psA[:],
                            mybir.ActivationFunctionType.Exp,
                            scale=scale,
                            accum_out=sum_h[:],
                        )
                        sum_parts.append(sum_h)
                    inv_t = p_stat.tile([P, 1], FP32, tag="inv")
                    if n_ah == 1:
                        nc.vector.reciprocal(inv_t[:], sum_parts[0][:])
                    else:
                        sum_t = p_stat.tile([P, 1], FP32, tag="sumt")
                        nc.vector.tensor_add(sum_t[:], sum_parts[0][:], sum_parts[1][:])
                        nc.vector.reciprocal(inv_t[:], sum_t[:])
                    inv_tiles[t] = inv_t
                    attn_sb = p_attn.tile([P, Sb], FP32, tag="attn")
                    nc.vector.tensor_scalar_mul(attn_sb[:], expA[:], inv_t[:])
                    nc.sync.dma_start(attn[b][ds(P * t, P), :], attn_sb[:])

                # ---- phase B: transposed scores + PV matmul for this chunk ----
                outT = ps_small.tile([P, 512], FP32, tag="pss")
                for sp in range(n_sp):
                    psT = ps_big.tile([P, 1024], FP32, tag="big")
                    expB = p_expB.tile([P, 1024], mm_dtype, tag="expB")
                    for i in range(2):
                        s = 2 * sp + i
                        nc.tensor.matmul(
                            psT[:, ds(512 * i, 512)],
                            kT[:, ts(s, P)],
                            qT[:, ds(512 * c, 512)],
                            start=True,
                            stop=True,
                        )
                    nc.scalar.activation(
                        expB[:],
                        psT[:],
                        mybir.ActivationFunctionType.Exp,
                        scale=scale,
                    )
                    for i in range(2):
                        s = 2 * sp + i
                        nc.tensor.matmul(
                            outT[:],
                            v_r[:, s, :],
                            expB[:, ds(512 * i, 512)],
                            start=(s == 0),
                            stop=(s == n_st - 1),
                        )
                outT_sb = p_small.tile([P, 512], FP32, tag="outTsb")
                nc.vector.tensor_copy(outT_sb[:], outT[:])
                out_c = p_small.tile([P, 4, DV], FP32, tag="outc")
                for t4 in range(4):
                    tr = ps_small.tile([P, 512], FP32, tag="pss")
                    nc.tensor.transpose(tr[:, 0:P], outT_sb[:, ts(t4, P)], identity32[:])
                    nc.vector.tensor_scalar_mul(
                        out_c[:, t4, :], tr[:, 0:P], inv_tiles[4 * c + t4][:]
                    )
                nc.sync.dma_start(
                    out[b].rearrange("(t p) d -> p t d", p=P)[:, ds(4 * c, 4), :],
                    out_c[:],
                )


def build_module(n_b=B_PER_CORE, Lb=L_FULL, Sb=S_FULL, enable_asserts=False,
                 mm_dtype=MM_DTYPE):
    nc = bacc.Bacc(
        "TRN2",
        target_bir_lowering=False,
        debug=False,
        enable_asserts=enable_asserts,
        num_devices=N_CORES,
    )
    q = nc.dram_tensor("q", (n_b, Lb, DK), FP32, kind="ExternalInput").ap()
    k = nc.dram_tensor("k", (n_b, Sb, DK), FP32, kind="ExternalInput").ap()
    v = nc.dram_tensor("v", (n_b, Sb, DV), FP32, kind="ExternalInput").ap()
    attn = nc.dram_tensor("attn", (n_b, Lb, Sb), FP32, kind="ExternalOutput").ap()
    out = nc.dram_tensor("out", (n_b, Lb, DV), FP32, kind="ExternalOutput").ap()
    with tile.TileContext(nc) as tc:
        build_attention_body(tc, q, k, v, attn, out, mm_dtype=mm_dtype)
    nc.compile()
    return nc


_CACHED_NC = None


def _get_module():
    global _CACHED_NC
    if _CACHED_NC is None:
        _CACHED_NC = build_module()
    return _CACHED_NC


def kernel(query, key, value):
    from concourse.bass_utils import run_bass_kernel_spmd

    query = np.ascontiguousarray(np.asarray(query, dtype=np.float32))
    key = np.ascontiguousarray(np.asarray(key, dtype=np.float32))
    value = np.ascontiguousarray(np.asarray(value, dtype=np.float32))

    nc = _get_module()
    in_maps = []
    for core in range(N_CORES):
        sl = slice(core * B_PER_CORE, (core + 1) * B_PER_CORE)
        in_maps.append({"q": query[sl], "k": key[sl], "v": value[sl]})
    res = run_bass_kernel_spmd(nc, in_maps, core_ids=list(range(N_CORES)))
    out = np.concatenate([r["out"] for r in res.results], axis=0)
    attn = np.concatenate([r["attn"] for r in res.results], axis=0)
    return out, attn


# revision 18
# speedup vs baseline: 2.1825x; 1.1114x over previous
"""Trainium2 Bass kernel for scaled dot-product attention.

Problem: B=16, L=S=2048, DK=DV=128, fp32.
reference: scores = (Q @ K^T) * 1/sqrt(DK); attn = softmax(scores, -1);
           out = attn @ V; returns (out, attn).

Sharding: data-parallel over batch, 2 batches per core on 8 NeuronCores.

Per-batch plan (matmuls in fp32r: 1 cycle/row vs 4 for fp32, inputs
rounded tf32-style, ~3e-4 rel error on outputs):
  - Load Q, K natural [128, nT, 128]; round to fp32r; PE-transpose tiles to
    build QT, KT [128(d), L] in SBUF (contraction dim on partitions).
  - Phase A (per 128-row l_tile): scores[l,s] = matmul(lhsT=QT tile,
    rhs=KT) -> PSUM; ACT exp(scale*x) with fused accum_out row-sums;
    DVE reciprocal; DVE tensor_scalar normalize (2x mode) in place;
    contiguous 1 MiB DMA store of attn rows.
  - Phase B (per 512-col l_chunk): scoresT[s,l] = matmul(lhsT=KT tile,
    rhs=QT chunk); ACT exp; PV matmul with V stationary accumulating
    outT[v, l] in a pinned PSUM bank over all 16 s-tiles; PE-transpose
    outT back to natural rows, normalize with phase-A's 1/rowsum, store.
  - Optional exact_a: 3-term fp32r error-corrected phase-A scores
    (q = q_r + q_err split) making stored attn fp32-exact at +PE cost.
"""

import math

import numpy as np

import concourse.bass as bass
import concourse.tile as tile
from concourse import bacc, mybir
from concourse.bass import ds, ts
from concourse.masks import make_identity

FP32 = mybir.dt.float32
FP32R = mybir.dt.float32r
P = 128
B_FULL = 16
N_CORES = 8
B_PER_CORE = B_FULL // N_CORES
L_FULL = 2048
S_FULL = 2048
DK = 128
DV = 128

MM_DTYPE = FP32R  # FP32 for exact (4x slower matmuls), FP32R for fast
EXACT_A = False   # 3-term fp32r error-corrected phase-A scores (exact attn)


def build_attention_body(tc, q, k, v, attn, out, mm_dtype=MM_DTYPE,
                         exact_a=EXACT_A):
    nc = tc.nc
    n_b, Lb, _ = q.shape
    Sb = k.shape[1]
    scale = 1.0 / math.sqrt(DK)

    n_lt = Lb // P          # 128-row l tiles
    n_st = Sb // P          # 128-row s tiles
    n_lc = Lb // 512        # 512-col l chunks (phase B)
    a_half = min(1024, Sb)  # phase A PSUM tile width
    n_ah = Sb // a_half
    n_sp = (n_st + 1) // 2  # phase B s-tile pairs

    rounded = mm_dtype != FP32
    exact_a = exact_a and rounded

    with (
        tc.tile_pool(name="consts", bufs=1) as consts,
        tc.tile_pool(name="p_nat", bufs=1) as p_nat,
        tc.tile_pool(name="p_vr", bufs=2) as p_vr,
        tc.tile_pool(name="p_qkT", bufs=2) as p_qkT,
        tc.tile_pool(name="p_qkTe", bufs=2) as p_qkTe,
        tc.tile_pool(name="p_expA", bufs=3) as p_expA,
        tc.tile_pool(name="p_expB", bufs=3) as p_expB,
        tc.tile_pool(name="p_small", bufs=2) as p_small,
        tc.tile_pool(name="p_stat", bufs=12) as p_stat,
        tc.tile_pool(name="ps_big", bufs=3, space="PSUM") as ps_big,
        tc.tile_pool(name="ps_small", bufs=2, space="PSUM") as ps_small,
    ):
        identity32 = consts.tile([P, P], FP32, tag="id32")
        make_identity(nc, identity32[:])
        if mm_dtype != FP32:
            identity = consts.tile([P, P], mm_dtype, tag="idr")
            nc.vector.tensor_copy(identity[:], identity32[:])
        else:
            identity = identity32

        def build_T(src, n_t, tag, pool, copy_eng=None):
            # 4 PE transposes per PSUM slot; copy each slice out as soon
            # as its transposes land so early columns unblock matmuls.
            # copy_eng="act" routes copies to ScalarE (idle at batch
            # start while DVE runs the rounding chain).
            dst = pool.tile([P, n_t * P], mm_dtype, tag=tag)
            for g in range(0, n_t, 4):
                gn = min(4, n_t - g)
                tr = ps_small.tile([P, 512], mm_dtype, tag="pss")
                for t in range(g, g + gn):
                    nc.tensor.transpose(
                        tr[:, ts(t - g, P)], src[:, t, :], identity[:]
                    )
                sl_d = ds(g * P, gn * P)
                sl_s = ds(0, gn * P)
                if copy_eng == "act":
                    nc.scalar.copy(dst[:, sl_d], tr[:, sl_s])
                else:
                    nc.vector.tensor_copy(dst[:, sl_d], tr[:, sl_s])
            return dst

        def prep_batch(b):
            # load Q/K, round, build transposed QT/KT (phase-B critical
            # path first), then V and residuals
            q_nat = p_nat.tile([P, n_lt, DK], FP32, tag="qnat")
            nc.sync.dma_start(q_nat[:], q[b].rearrange("(t p) d -> p t d", p=P))
            k_nat = p_nat.tile([P, n_st, DK], FP32, tag="knat")
            nc.sync.dma_start(k_nat[:], k[b].rearrange("(t p) d -> p t d", p=P))
            v_sb = p_nat.tile([P, n_st, DV], FP32, tag="vsb")
            nc.sync.dma_start(v_sb[:], v[b].rearrange("(t p) d -> p t d", p=P))

            if rounded:
                q_r = p_nat.tile([P, n_lt, DK], mm_dtype, tag="qr")
                nc.vector.tensor_copy(q_r[:], q_nat[:])
                k_r = p_nat.tile([P, n_st, DK], mm_dtype, tag="kr")
                nc.vector.tensor_copy(k_r[:], k_nat[:])
            else:
                q_r, k_r = q_nat, k_nat

            qT = build_T(q_r, n_lt, "qT", p_qkT)
            kT = build_T(k_r, n_st, "kT", p_qkT)
            qTe = kTe = None
            if rounded:
                v_r = p_vr.tile([P, n_st, DV], mm_dtype, tag="vr")
                nc.vector.tensor_copy(v_r[:], v_sb[:])
                if exact_a:
                    # fp32r rounding residuals (for exact phase-A scores):
                    # q = q_r + q_err; scores = qr.kr + qe.kr + qr.ke
                    # subtract in fp32, output dtype fp32r rounds in one op
                    q_e = p_nat.tile([P, n_lt, DK], mm_dtype, tag="qe")
                    nc.vector.tensor_sub(q_e[:], q_nat[:], q_r[:].bitcast(FP32))
                    k_e = p_nat.tile([P, n_st, DK], mm_dtype, tag="ke")
                    nc.vector.tensor_sub(k_e[:], k_nat[:], k_r[:].bitcast(FP32))
                    qTe = build_T(q_e, n_lt, "qTe", p_qkTe, copy_eng="act")
                    kTe = build_T(k_e, n_st, "kTe", p_qkTe, copy_eng="act")
            else:
                v_r = v_sb
            return qT, kT, qTe, kTe, v_r

        prep = prep_batch(0)
        for b in range(n_b):
            qT, kT, qTe, kTe, v_r = prep
            inv_tiles = {}

            def emit_B_pair(c, sp, outT):
                psT = ps_big.tile([P, 1024], FP32, tag="big")
                expB = p_expB.tile([P, 1024], mm_dtype, tag="expB")
                gn = min(2, n_st - 2 * sp)
                for i in range(gn):
                    s = 2 * sp + i
                    nc.tensor.matmul(
                        psT[:, ds(512 * i, 512)],
                        kT[:, ts(s, P)],
                        qT[:, ds(512 * c, 512)],
                        start=True,
                        stop=True,
                    )
                nc.scalar.activation(
                    expB[:, ds(0, gn * 512)],
                    psT[:, ds(0, gn * 512)],
                    mybir.ActivationFunctionType.Exp,
                    scale=scale,
                )
                for i in range(gn):
                    s = 2 * sp + i
                    nc.tensor.matmul(
                        outT[:],
                        v_r[:, s, :],
                        expB[:, ds(512 * i, 512)],
                        start=(s == 0),
                        stop=(s == n_st - 1),
                    )

            def emit_A_tile(t):
                expA = p_expA.tile([P, Sb], FP32, tag="expA")
                sum_parts = []
                for h in range(n_ah):
                    psA = ps_big.tile([P, a_half], FP32, tag="big")
                    for j in range(a_half // 512):
                        sl = ds(a_half * h + 512 * j, 512)
                        terms = [(qT, kT)]
                        if exact_a:
                            terms += [(qTe, kT), (qT, kTe)]
                        for ti, (lh, rh) in enumerate(terms):
                            nc.tensor.matmul(
                                psA[:, ds(512 * j, 512)],
                                lh[:, ts(t, P)],
                                rh[:, sl],
                                start=(ti == 0),
                                stop=(ti == len(terms) - 1),
                            )
                    sum_h = p_stat.tile([P, 1], FP32, tag="sumh")
                    nc.scalar.activation(
                        expA[:, ds(a_half * h, a_half)],
                        psA[:],
                        mybir.ActivationFunctionType.Exp,
                        scale=scale,
                        accum_out=sum_h[:],
                    )
                    sum_parts.append(sum_h)
                inv_t = p_stat.tile([P, 1], FP32, tag="inv")
                if n_ah == 1:
                    nc.vector.reciprocal(inv_t[:], sum_parts[0][:])
                else:
                    sum_t = p_stat.tile([P, 1], FP32, tag="sumt")
                    nc.vector.tensor_add(sum_t[:], sum_parts[0][:], sum_parts[1][:])
                    nc.vector.reciprocal(inv_t[:], sum_t[:])
                inv_tiles[t] = inv_t
                nc.vector.tensor_scalar_mul(expA[:], expA[:], inv_t[:])
                nc.sync.dma_start(attn[b][ds(P * t, P), :], expA[:])

            for c in range(n_lc):
                # prefetch next batch's inputs/transposes while this batch
                # still has compute left to hide them behind
                if c == max(0, n_lc - 2) and b + 1 < n_b:
                    prep = prep_batch(b + 1)
                # interleave phase-B pairs with phase-A tiles so PSUM slot
                # demand and PE/ACT work stay spread across the chunk
                outT = ps_small.tile([P, 512], FP32, tag="pss")
                for t4 in range(4):
                    for sp in range(t4 * n_sp // 4, (t4 + 1) * n_sp // 4):
                        emit_B_pair(c, sp, outT)
                    emit_A_tile(4 * c + t4)

                # ---- phase B tail: transpose outT to natural rows + store ----
                outT_sb = p_small.tile([P, 512], FP32, tag="outTsb")
                nc.vector.tensor_copy(outT_sb[:], outT[:])
                out_c = p_small.tile([P, 4, DV], FP32, tag="outc")
                for t4 in range(4):
                    tr = ps_small.tile([P, 512], FP32, tag="pss")
                    nc.tensor.transpose(
                        tr[:, 0:P], outT_sb[:, ts(t4, P)], identity32[:]
                    )
                    nc.vector.tensor_scalar_mul(
                        out_c[:, t4, :], tr[:, 0:P], inv_tiles[4 * c + t4][:]
                    )
                nc.sync.dma_start(
                    out[b].rearrange("(t p) d -> p t d", p=P)[:, ds(4 * c, 4), :],
                    out_c[:],
                )


def build_module(n_b=B_PER_CORE, Lb=L_FULL, Sb=S_FULL, enable_asserts=False,
                 mm_dtype=MM_DTYPE, exact_a=None):
    nc = bacc.Bacc(
        "TRN2",
        target_bir_lowering=False,
        debug=False,
        enable_asserts=enable_asserts,
        num_devices=N_CORES,
    )
    q = nc.dram_tensor("q", (n_b, Lb, DK), FP32, kind="ExternalInput").ap()
    k = nc.dram_tensor("k", (n_b, Sb, DK), FP32, kind="ExternalInput").ap()
    v = nc.dram_tensor("v", (n_b, Sb, DV), FP32, kind="ExternalInput").ap()
    attn = nc.dram_tensor("attn", (n_b, Lb, Sb), FP32, kind="ExternalOutput").ap()
    out = nc.dram_tensor("out", (n_b, Lb, DV), FP32, kind="ExternalOutput").ap()
    if exact_a is None:
        exact_a = EXACT_A
    with tile.TileContext(nc) as tc:
        build_attention_body(tc, q, k, v, attn, out, mm_dtype=mm_dtype,
                             exact_a=exact_a)
    nc.compile()
    return nc


_CACHED_NC = None


def _get_module():
    global _CACHED_NC
    if _CACHED_NC is None:
        _CACHED_NC = build_module()
    return _CACHED_NC


def kernel(query, key, value):
    from concourse.bass_utils import run_bass_kernel_spmd

    query = np.ascontiguousarray(np.asarray(query, dtype=np.float32))
    key = np.ascontiguousarray(np.asarray(key, dtype=np.float32))
    value = np.ascontiguousarray(np.asarray(value, dtype=np.float32))

    nc = _get_module()
    in_maps = []
    for core in range(N_CORES):
        sl = slice(core * B_PER_CORE, (core + 1) * B_PER_CORE)
        in_maps.append({"q": query[sl], "k": key[sl], "v": value[sl]})
    res = run_bass_kernel_spmd(nc, in_maps, core_ids=list(range(N_CORES)))
    out = np.concatenate([r["out"] for r in res.results], axis=0)
    attn = np.concatenate([r["attn"] for r in res.results], axis=0)
    return out, attn
